# revision 10
# baseline (speedup 1.0000x reference)
"""MiMoV2 flash-attention block on 8 Trainium2 NeuronCores.

Sharding: tensor-parallel over heads. Core c owns Q heads [4c, 4c+4) and
KV head c//2 (GQA group-aligned, so no KV exchange is needed). Each core
computes its heads' projections + attention + a row-parallel slice of the
output projection; the host sums the 8 partial outputs.

Layouts are chosen so no on-device transposes are needed:
  - host feeds hidden_states transposed (hsT [hidden, B*S]) and weights
    pre-transposed, all in bf16
  - Q/K are produced feature-major ([dq, tok]); V token-major ([tok, dv])
  - score tiles are key-major [j, q]: softmax denominator via a ones-matmul,
    normalization deferred to after the P@V matmul
"""

import sys

sys.path.insert(0, "/opt/trn_rl_repo")

import math

import numpy as np

import concourse.bass as bass
import concourse.tile as tile
from concourse import mybir
from concourse.bass_utils import run_bass_kernel_spmd

try:
    import ml_dtypes

    BF16 = ml_dtypes.bfloat16
except ImportError:  # pragma: no cover
    import jax.numpy as jnp

    BF16 = jnp.bfloat16

# ---- model constants (hardcoded per problem spec) ----
HIDDEN = 4096
N_HEADS = 32
N_KV_HEADS = 4
QK_DIM = 192
V_DIM = 128
ROT = 64
ROPE_THETA = 5000000.0
B = 2
S = 2048
NCORES = 8
HL = N_HEADS // NCORES  # 4 local Q heads
KT = HIDDEN // 128  # 32 contraction tiles
CH = 512  # token chunk for projections
NCH = S // CH  # 4 chunks per batch
QC = 512  # query chunk for attention
JT = S // 128  # 16 key tiles per batch
SCALE = 1.0 / math.sqrt(QK_DIM)
NEG = -1e9

f32 = mybir.dt.float32
bf16 = mybir.dt.bfloat16


def split_multi_waits(nc: bass.Bass) -> None:
    """The walrus build in this container accepts at most one sync-wait per
    instruction (CoreV3 setupSyncWait). Tile's sem-assignment attaches several.
    Hoist all but the last wait of each instruction onto single-wait NOPs
    inserted just before it — engines execute their queue in order, so the
    semantics are identical."""
    f = nc.m.functions[0]
    for blk in f.blocks:
        insts = blk.instructions
        out = []
        changed = False
        for inst in insts:
            si = inst.sync_info
            if si is not None and len(si.on_wait) > 1:
                waits = list(si.on_wait)
                mk = type(si)
                for i, w in enumerate(waits[:-1]):
                    out.append(
                        mybir.InstNoOp(
                            name=f"{inst.name}.w{i}",
                            engine=inst.engine,
                            sync_info=mk(on_wait=[w], on_update=[]),
                            bass_nofuse=True,
                        )
                    )
                inst.sync_info = mk(
                    on_wait=[waits[-1]], on_update=list(si.on_update)
                )
                changed = True
            out.append(inst)
        if changed:
            blk.instructions = out


def build_program(mode: str) -> bass.Bass:
    """mode: 'causal' (skip masked tiles), 'none' (no mask), 'general'
    (additive mask tiles loaded from DRAM)."""
    nc = bass.Bass()

    hsT = nc.declare_dram_parameter("hsT", [HIDDEN, B * S], bf16, isOutput=False)
    wqT = nc.declare_dram_parameter("wqT", [HIDDEN, HL * QK_DIM], bf16, isOutput=False)
    wkT = nc.declare_dram_parameter("wkT", [HIDDEN, QK_DIM], bf16, isOutput=False)
    wvT = nc.declare_dram_parameter("wvT", [HIDDEN, V_DIM], bf16, isOutput=False)
    woT = nc.declare_dram_parameter("woT", [HL * V_DIM, HIDDEN], bf16, isOutput=False)
    rope = nc.declare_dram_parameter("rope", [ROT, 2, B * S], bf16, isOutput=False)
    diag = nc.declare_dram_parameter("diag", [128, 896], f32, isOutput=False)
    if mode == "general":
        maskT = nc.declare_dram_parameter("maskT", [B, S, S], bf16, isOutput=False)
    y = nc.declare_dram_parameter("y", [B, S, HIDDEN], f32, isOutput=True)

    hsT_t = hsT.rearrange("(kt p) t -> p kt t", p=128)
    wqT_t = wqT.rearrange("(kt p) n -> p kt n", p=128)
    wkT_t = wkT.rearrange("(kt p) n -> p kt n", p=128)
    wvT_t = wvT.rearrange("(kt p) n -> p kt n", p=128)
    woT_t = woT.rearrange("(h p) n -> p h n", p=128)

    Exp = mybir.ActivationFunctionType.Exp

    with tile.TileContext(nc) as tc:
        with (
            tc.tile_pool(name="singles", bufs=1) as singles,
            tc.tile_pool(name="hsc", bufs=1) as hsc_pool,
            tc.tile_pool(name="qkv", bufs=1) as qkv_pool,
            tc.tile_pool(name="ropep", bufs=1) as rope_pool,
            tc.tile_pool(name="rtmp", bufs=2) as rtmp_pool,
            tc.tile_pool(name="epool", bufs=2) as e_pool,
            tc.tile_pool(name="ao", bufs=2) as ao_pool,
            tc.tile_pool(name="wot", bufs=2) as wot_pool,
            tc.tile_pool(name="ysb", bufs=2) as y_pool,
            tc.tile_pool(name="norm", bufs=2) as norm_pool,
            tc.tile_pool(name="mtiles", bufs=2) as m_pool,
            tc.tile_pool(name="ps512", bufs=3, space="PSUM") as ps512,
            tc.tile_pool(name="psacc", bufs=3, space="PSUM") as psacc,
            tc.tile_pool(name="psd", bufs=1, space="PSUM") as psd_pool,
        ):
            # ---- resident constants / weights ----
            wq_sb = singles.tile([128, KT, HL * QK_DIM], bf16)
            nc.sync.dma_start(out=wq_sb, in_=wqT_t)
            wk_sb = singles.tile([128, KT, QK_DIM], bf16)
            nc.sync.dma_start(out=wk_sb, in_=wkT_t)
            wv_sb = singles.tile([128, KT, V_DIM], bf16)
            nc.sync.dma_start(out=wv_sb, in_=wvT_t)
            diag_sb = singles.tile([128, 896], f32)
            nc.sync.dma_start(out=diag_sb, in_=diag[:, :])
            ones_sb = singles.tile([128, 128], bf16)
            nc.vector.memset(ones_sb, 1.0)

            for b in range(B):
                rope_sb = rope_pool.tile([ROT, 2, S], bf16)
                nc.sync.dma_start(out=rope_sb, in_=rope[:, :, b * S : (b + 1) * S])

                def do_rope(ps, dst, sl):
                    """Partial RoPE on ps[0:64] (feature-major f32 psum),
                    bf16 result into dst (sbuf, 64 partitions)."""
                    cos = rope_sb[:, 0, sl]
                    sinS = rope_sb[:, 1, sl]
                    rt = rtmp_pool.tile([ROT, 2, CH], bf16, tag="rope")
                    tmp, tmp2 = rt[:, 0, :], rt[:, 1, :]
                    # sinS rows 0:32 hold -sin, rows 32:64 hold +sin
                    nc.vector.tensor_mul(tmp[0:32], ps[32:64], sinS[0:32])
                    nc.vector.tensor_mul(tmp[32:64], ps[0:32], sinS[32:64])
                    nc.vector.tensor_mul(tmp2, ps[0:64], cos)
                    nc.vector.tensor_add(dst, tmp, tmp2)

                qa_sb = qkv_pool.tile([128, HL, S], bf16, tag="qa")
                qb_sb = qkv_pool.tile([128, HL // 2, S], bf16, tag="qb")
                ka_sb = qkv_pool.tile([128, S], bf16, tag="ka")
                # kb lives at partitions 0:64; a duplicate at 64:128 lets the
                # odd heads' B-matmul (whose Q half sits at partitions 64:128
                # of the packed pair tile) use matching base partitions.
                kb_sb = qkv_pool.tile([128, S], bf16, tag="kb")
                v_sb = qkv_pool.tile([128, JT, V_DIM], bf16, tag="v")

                # ================= projections =================
                for ch in range(NCH):
                    t0 = b * S + ch * CH
                    sl = slice(ch * CH, ch * CH + CH)
                    hsc = hsc_pool.tile([128, KT, CH], bf16)
                    nc.sync.dma_start(out=hsc, in_=hsT_t[:, :, t0 : t0 + CH])

                    for h in range(HL):
                        ps = ps512.tile([128, CH], f32, tag="ps")
                        for k in range(KT):
                            nc.tensor.matmul(
                                ps,
                                lhsT=wq_sb[:, k, h * 128 : h * 128 + 128],
                                rhs=hsc[:, k, :],
                                start=(k == 0),
                                stop=(k == KT - 1),
                            )
                        do_rope(ps, qa_sb[0:64, h, sl], sl)
                        nc.scalar.copy(qa_sb[64:128, h, sl], ps[64:128])

                    for p2 in range(HL // 2):
                        ps = ps512.tile([128, CH], f32, tag="ps")
                        for k in range(KT):
                            nc.tensor.matmul(
                                ps,
                                lhsT=wq_sb[
                                    :, k, HL * 128 + p2 * 128 : HL * 128 + p2 * 128 + 128
                                ],
                                rhs=hsc[:, k, :],
                                start=(k == 0),
                                stop=(k == KT - 1),
                            )
                        nc.scalar.copy(qb_sb[:, p2, sl], ps)

                    ps = ps512.tile([128, CH], f32, tag="ps")
                    for k in range(KT):
                        nc.tensor.matmul(
                            ps,
                            lhsT=wk_sb[:, k, 0:128],
                            rhs=hsc[:, k, :],
                            start=(k == 0),
                            stop=(k == KT - 1),
                        )
                    do_rope(ps, ka_sb[0:64, sl], sl)
                    nc.scalar.copy(ka_sb[64:128, sl], ps[64:128])

                    psb = ps512.tile([64, CH], f32, tag="ps")
                    for k in range(KT):
                        nc.tensor.matmul(
                            psb,
                            lhsT=wk_sb[:, k, 128:QK_DIM],
                            rhs=hsc[:, k, :],
                            start=(k == 0),
                            stop=(k == KT - 1),
                        )
                    nc.scalar.copy(kb_sb[0:64, sl], psb)
                    nc.sync.dma_start(out=kb_sb[64:128, sl], in_=kb_sb[0:64, sl])

                    for tt in range(CH // 128):
                        psv = ps512.tile([128, V_DIM], f32, tag="ps")
                        for k in range(KT):
                            nc.tensor.matmul(
                                psv,
                                lhsT=hsc[:, k, tt * 128 : tt * 128 + 128],
                                rhs=wv_sb[:, k, :],
                                start=(k == 0),
                                stop=(k == KT - 1),
                            )
                        nc.vector.tensor_copy(v_sb[:, ch * 4 + tt, :], psv)

                # ================= attention + out-proj =================
                if mode == "general":
                    maskT_t = maskT[b].rearrange("(jt p) q -> p jt q", p=128)

                for qc in range(S // QC):
                    jmax = (qc + 1) * (QC // 128) if mode == "causal" else JT
                    qsl = slice(qc * QC, qc * QC + QC)

                    if mode == "general":
                        mq = m_pool.tile([128, JT, QC], bf16, tag="mq")
                        nc.sync.dma_start(out=mq, in_=maskT_t[:, :, qsl])

                    ao_sb = ao_pool.tile([128, HL, QC], bf16)
                    for h in range(HL):
                        pso = psacc.tile([128, QC], f32, tag="acc")
                        psd = psd_pool.tile([1, QC], f32, tag="d")
                        qbb = 0 if h % 2 == 0 else 64
                        for jt in range(jmax):
                            jsl = slice(jt * 128, jt * 128 + 128)
                            pss = ps512.tile([128, QC], f32, tag="ps")
                            nc.tensor.matmul(
                                pss,
                                lhsT=ka_sb[:, jsl],
                                rhs=qa_sb[:, h, qsl],
                                start=True,
                                stop=False,
                            )
                            nc.tensor.matmul(
                                pss,
                                lhsT=kb_sb[qbb : qbb + 64, jsl],
                                rhs=qb_sb[qbb : qbb + 64, h // 2, qsl],
                                start=False,
                                stop=True,
                            )
                            if mode == "causal" and jt >= qc * (QC // 128):
                                r = (jt - qc * (QC // 128)) * 128
                                nc.vector.tensor_add(
                                    pss, pss, diag_sb[:, 384 - r : 896 - r]
                                )
                            elif mode == "general":
                                nc.vector.tensor_add(pss, pss, mq[:, jt, :])
                            e = e_pool.tile([128, QC], bf16)
                            nc.scalar.activation(e, pss, Exp, scale=SCALE)
                            nc.tensor.matmul(
                                psd,
                                lhsT=ones_sb[:, 0:1],
                                rhs=e,
                                start=(jt == 0),
                                stop=(jt == jmax - 1),
                            )
                            nc.tensor.matmul(
                                pso,
                                lhsT=v_sb[:, jt, :],
                                rhs=e,
                                start=(jt == 0),
                                stop=(jt == jmax - 1),
                            )
                        bc = norm_pool.tile([128, QC], bf16, tag="bc")
                        with nc.allow_low_precision(
                            reason="bf16 softmax denominators, matches bf16 attn"
                        ):
                            nc.vector.reciprocal(bc[0:1, :], psd)
                        psbc = psd_pool.tile([128, QC], f32, tag="bc")
                        nc.tensor.matmul(
                            psbc,
                            lhsT=ones_sb[0:1, :],
                            rhs=bc[0:1, :],
                            start=True,
                            stop=True,
                        )
                        nc.scalar.copy(bc, psbc)
                        nc.vector.tensor_mul(ao_sb[:, h, :], pso, bc)

                    # out-projection for this query chunk
                    for ncol in range(HIDDEN // 512):
                        wo_t = wot_pool.tile([128, HL, 512], bf16)
                        nc.sync.dma_start(
                            out=wo_t,
                            in_=woT_t[:, :, ncol * 512 : ncol * 512 + 512],
                        )
                        for tt in range(QC // 128):
                            psy = psacc.tile([128, 512], f32, tag="acc")
                            for h in range(HL):
                                nc.tensor.matmul(
                                    psy,
                                    lhsT=ao_sb[:, h, tt * 128 : tt * 128 + 128],
                                    rhs=wo_t[:, h, :],
                                    start=(h == 0),
                                    stop=(h == HL - 1),
                                )
                            ysb = y_pool.tile([128, 512], f32)
                            nc.vector.tensor_copy(ysb, psy)
                            nc.sync.dma_start(
                                out=y[
                                    b,
                                    qc * QC + tt * 128 : qc * QC + tt * 128 + 128,
                                    ncol * 512 : ncol * 512 + 512,
                                ],
                                in_=ysb,
                            )
    split_multi_waits(nc)
    return nc


_PROGRAM_CACHE: dict = {}


def _get_program(mode: str) -> bass.Bass:
    if mode not in _PROGRAM_CACHE:
        _PROGRAM_CACHE[mode] = build_program(mode)
    return _PROGRAM_CACHE[mode]


def _pack_wq(wq_c):
    """Permute the per-core Q weight rows so the transposed tile has columns
    [A0|A1|A2|A3|B01|B23]: A = dims 0:128 of each head, B = dims 128:192 of
    head pairs packed contiguously (so each matmul lhsT is one free-dim
    slice)."""
    rows = []
    for h in range(HL):
        rows.append(wq_c[h * QK_DIM : h * QK_DIM + 128])
    for h in range(HL):
        rows.append(wq_c[h * QK_DIM + 128 : (h + 1) * QK_DIM])
    packed = np.concatenate(rows, axis=0)  # [768, HIDDEN]
    return np.ascontiguousarray(packed.T).astype(BF16)


def _host_inputs(hidden_states, attention_mask, position_ids, wq, wk, wv, wo):
    hs2 = np.asarray(hidden_states, np.float32).reshape(B * S, HIDDEN)
    hsT = np.ascontiguousarray(hs2.T).astype(BF16)

    # mode detection
    mask = np.asarray(attention_mask, np.float32)
    causal = np.triu(np.full((S, S), NEG, np.float32), k=1)
    if np.array_equal(mask, np.broadcast_to(causal[None, None], mask.shape)):
        mode = "causal"
    elif not mask.any():
        mode = "none"
    else:
        mode = "general"

    # rope tables, feature-major over global tokens
    pos = np.asarray(position_ids).astype(np.float32)  # [B, S]
    inv_freq = (
        1.0 / (ROPE_THETA ** (np.arange(0, ROT, 2, dtype=np.float32) / ROT))
    ).astype(np.float32)
    freqs = pos[:, :, None] * inv_freq[None, None, :]  # [B, S, 32]
    cos_h = np.cos(freqs).reshape(B * S, ROT // 2).T  # [32, B*S]
    sin_h = np.sin(freqs).reshape(B * S, ROT // 2).T
    rope_t = np.empty((ROT, 2, B * S), np.float32)
    rope_t[0:32, 0] = cos_h
    rope_t[32:64, 0] = cos_h
    rope_t[0:32, 1] = -sin_h
    rope_t[32:64, 1] = sin_h
    rope_t = rope_t.astype(BF16)

    # causal diagonal-tile mask table: allowed iff j <= x - 384
    jj = np.arange(128)[:, None]
    xx = np.arange(896)[None, :]
    diag_t = np.where(jj <= xx - 384, 0.0, NEG).astype(np.float32)

    wq_ = np.asarray(wq, np.float32)
    wk_ = np.asarray(wk, np.float32)
    wv_ = np.asarray(wv, np.float32)
    wo_ = np.asarray(wo, np.float32)

    in_maps = []
    for c in range(NCORES):
        kv = c // 2
        d = {
            "hsT": hsT,
            "wqT": _pack_wq(wq_[c * HL * QK_DIM : (c + 1) * HL * QK_DIM, :]),
            "wkT": np.ascontiguousarray(
                wk_[kv * QK_DIM : (kv + 1) * QK_DIM, :].T
            ).astype(BF16),
            "wvT": np.ascontiguousarray(
                wv_[kv * V_DIM : (kv + 1) * V_DIM, :].T
            ).astype(BF16),
            "woT": np.ascontiguousarray(
                wo_[:, c * HL * V_DIM : (c + 1) * HL * V_DIM].T
            ).astype(BF16),
            "rope": rope_t,
            "diag": diag_t,
        }
        if mode == "general":
            scaled = mask[:, 0].transpose(0, 2, 1) / SCALE  # [B, j, q], pre-descale
            d["maskT"] = np.ascontiguousarray(scaled).astype(BF16)
        in_maps.append(d)
    return mode, in_maps


def kernel(
    hidden_states, attention_mask, position_ids, wq, wk, wv, wo, _trace=False
):
    mode, in_maps = _host_inputs(
        hidden_states, attention_mask, position_ids, wq, wk, wv, wo
    )
    nc = _get_program(mode)
    res = run_bass_kernel_spmd(nc, in_maps, list(range(NCORES)), trace=_trace)
    out = res.results[0]["y"].astype(np.float32)
    for c in range(1, NCORES):
        out += res.results[c]["y"]
    if _trace:
        kernel._last_result = res
    return out
